# revision 21
# baseline (speedup 1.0000x reference)
"""Trainium2 Bass kernel for nn_AttnGreedySearch (attn greedy top-1 search).

Math restructure (exact in exact arithmetic):
  With A_t = W_k^t and c_t = b_k @ sum_{i<t} W_k^i (row form), the iterated
  corpus is ic_t = ic0 @ A_t + c_t where ic0 = X @ W_proj + b_proj.  The
  device works with the bias-free ic0' = X @ W_proj and a DEFICIT-FREE u~
  recurrence u~_{j+1} = W_k u~_j + M_j g_j (no constant add): the dropped
  constants dv'_j accumulate into a sample-independent deficit
  D_{j+1} = W_k D_j + dv'_j whose score contribution <x W_proj, D_j> is
  emitted by the SAME projection matmul through 4 spare stationary columns
  (e_j), added to the scores before the argmax.  b_proj similarly folds into
  cv'_j / dv'_j.  Per iteration j:
      score_j[s,i] = <ic0'[s,i,:], u~_j> + e_j[s,i]   (argmax-equivalent)
      g_j = ic0'[argmax]                               (one-hot select)
      v_j = A_{j+1}^T g_j + cv'_j                      (output row, exact)
      u~_{j+1} = W_k u~_j + M_j g_j,   M_j = A_{j+2} A_{j+1}^T
  The 262MB corpus is read once, compressed by the projection, and the
  whole recurrence runs on 16-dim per-sample state.

Device dataflow (per core, batch B; super-tiles ST of 512 samples):
  P1 per tile: DMA X [128,1000]; PE-transpose item blocks -> psum; fat
      strided copies assemble d-major xt_sb [100, 10*512] (ACT+DVE).
  P1b per ST: projection as N=512 fp32 matmuls (M=32: 16 ic0 cols + 4
      e-cols), 4 items per psum bank at 32-partition offsets; copy
      psum->SBUF (ACT); PE-transpose [128,128] chunks back to sample-major;
      4-level-AP copies assemble ic0a (ACT), e_sb (ACT), and ic0b derived
      from ic0a on GPSIMD (SBUF->SBUF).
  P2: groups of 1024 samples processed in PAIRS with iterations interleaved:
      score = mul+grouped-reduce + e-add (DVE), one-hot mask (DVE), masked
      select-sum (GPSIMD mul + DVE reduce), 16x16 recurrence as pair-batched
      N=256 float32r block-diag matmuls (PE); copies on ACT.  P1 work is
      software-pipelined between P2 iteration steps at emission level so
      every engine FIFO alternates P1/P2 work.
  P3: per tile, one [128,96] DMA stores [user | v_0..v_4] rows.
"""

import numpy as np

import concourse.bass as bass
import concourse.mybir as mybir
import concourse.tile as tile
from concourse import bacc
from concourse.bass_utils import run_bass_kernel_spmd
from concourse.masks import make_identity

F32 = mybir.dt.float32
F32R = mybir.dt.float32r
SEARCH_NUM = 5
NCORES = 8
D = 100   # item feature dim
NSI = 10  # items per sample
H = 16    # projected dim
SH = NSI * H  # 160
NE = SEARCH_NUM - 1  # e-correction columns (j = 1..4)
SE = NSI * NE  # 40


def _host_constants(W_proj, b_proj, W_k, b_k):
    Wk = W_k.astype(np.float64)
    bk = b_k.astype(np.float64)
    bp = b_proj.astype(np.float64)
    Wp = W_proj.astype(np.float64)
    A = [np.eye(H)]
    for _ in range(SEARCH_NUM + 1):
        A.append(A[-1] @ Wk)
    c = [np.zeros(H)]
    for _ in range(SEARCH_NUM + 1):
        c.append(c[-1] @ Wk + bk)

    def blkdiag8(m):
        out = np.zeros((128, 128))
        for t in range(8):
            out[t * H:(t + 1) * H, t * H:(t + 1) * H] = m
        return out.astype(np.float32)

    cst = {"blk_wk": blkdiag8(Wk.T)}
    dvp = []
    for j in range(SEARCH_NUM):
        Aj1 = A[j + 1]
        cst[f"blk_a{j}"] = blkdiag8(Aj1)
        cv = bp @ Aj1 + c[j + 1]
        cst[f"cv{j}"] = np.tile(cv, 8).astype(np.float32)[:, None]
        if j < SEARCH_NUM - 1:
            Mj = A[j + 2] @ Aj1.T
            cst[f"blk_m{j}"] = blkdiag8(Mj.T)
            dvp.append(A[j + 2] @ c[j + 1] + Mj @ bp)
    # deficit recurrence D_{j+1} = Wk D_j + dv'_j  (D_0 = 0)
    Dd = [np.zeros(H)]
    for j in range(SEARCH_NUM - 1):
        Dd.append(Wk @ Dd[j] + dvp[j])
    waug = np.zeros((D, H + NE), dtype=np.float32)
    waug[:, :H] = W_proj.astype(np.float32)
    for j in range(1, SEARCH_NUM):
        waug[:, H + j - 1] = (Wp @ Dd[j]).astype(np.float32)
    cst["waug"] = waug
    return cst


def _v(t, off, dims, nparts=None):
    """View on tile/AP t: free dims `dims`, element offset `off` added.
    `nparts` overrides the partition count (step preserved)."""
    p = list(t.ap[0])
    if nparts is not None:
        p = [p[0], nparts]
    return bass.AP(tensor=t.tensor, offset=t.offset + off,
                   ap=[p] + [list(d) for d in dims])


def _R(x):
    """Bitcast an AP/tile to float32r."""
    if not isinstance(x, bass.AP):
        x = x.ap()
    return x.bitcast(F32R)


def _F(x):
    """Bitcast an AP/tile back to float32."""
    if not isinstance(x, bass.AP):
        x = x.ap()
    return x.bitcast(F32)


class _Slots:
    """Manual [128,128]-slot packing inside [128,512] PSUM bank tiles."""

    def __init__(self, pool, dt):
        self.pool, self.dt = pool, dt
        self.tile = None
        self.i = 4

    def get(self, w=1):
        if self.i + w > 4:
            self.tile = self.pool.tile([128, 512], self.dt, name="p2s",
                                       tag="p2s")
            self.i = 0
        v = _v(self.tile, self.i * 128, [[1, 128 * w]])
        self.i += w
        return v


def build_program(nc, B):
    assert B % 2048 == 0
    NT = B // 128
    NST = B // 512
    NG = B // 1024
    NW = NG // 2  # group pairs / pipeline windows
    dt = F32

    x_d = nc.dram_tensor("x", [B, NSI, D], F32R, kind="ExternalInput").ap()
    user_d = nc.dram_tensor("user", [B, H], dt, kind="ExternalInput").ap()
    waug_d = nc.dram_tensor("waug", [D, H + NE], dt, kind="ExternalInput").ap()
    blk_wk_d = nc.dram_tensor("blk_wk", [128, 128], F32R,
                              kind="ExternalInput").ap()
    blk_a_d = [nc.dram_tensor(f"blk_a{j}", [128, 128], F32R,
                              kind="ExternalInput").ap()
               for j in range(SEARCH_NUM)]
    cv_d = [nc.dram_tensor(f"cv{j}", [128, 1], dt, kind="ExternalInput").ap()
            for j in range(SEARCH_NUM)]
    blk_m_d = [nc.dram_tensor(f"blk_m{j}", [128, 128], F32R,
                              kind="ExternalInput").ap()
               for j in range(SEARCH_NUM - 1)]
    out_d = nc.dram_tensor("out", [B, SEARCH_NUM + 1, H], dt,
                           kind="ExternalOutput").ap()

    with tile.TileContext(nc) as tc:
        with tc.tile_pool(name="singles", bufs=1) as singles, \
             tc.tile_pool(name="xst", bufs=3) as xst, \
             tc.tile_pool(name="xtp", bufs=3) as xtp, \
             tc.tile_pool(name="scr", bufs=2) as scr, \
             tc.tile_pool(name="scrbig", bufs=1) as scrbig, \
             tc.tile_pool(name="vop", bufs=1) as vop, \
             tc.tile_pool(name="ptx", bufs=2, space="PSUM") as ptx, \
             tc.tile_pool(name="pbig", bufs=3, space="PSUM") as pbig, \
             tc.tile_pool(name="pp2", bufs=3, space="PSUM") as pp2:

            slots = _Slots(pp2, dt)

            # ---- persistent SBUF ----
            ident = singles.tile([128, 128], dt)
            make_identity(nc, ident)
            identR = singles.tile([128, 128], F32R)
            nc.scalar.copy(identR, ident)
            waug_sb = singles.tile([D, H + NE], dt)
            nc.sync.dma_start(out=waug_sb, in_=waug_d)
            blk_wk_sb = singles.tile([128, 128], F32R)
            nc.sync.dma_start(out=blk_wk_sb, in_=blk_wk_d)
            blk_a_sb, cv_sb, blk_m_sb = [], [], []
            for j in range(SEARCH_NUM):
                t_ = singles.tile([128, 128], F32R, name=f"blk_a{j}_sb")
                nc.sync.dma_start(out=t_, in_=blk_a_d[j])
                blk_a_sb.append(t_)
                t_ = singles.tile([128, 1], dt, name=f"cv{j}_sb")
                nc.sync.dma_start(out=t_, in_=cv_d[j])
                cv_sb.append(t_)
            for j in range(SEARCH_NUM - 1):
                t_ = singles.tile([128, 128], F32R, name=f"blk_m{j}_sb")
                nc.sync.dma_start(out=t_, in_=blk_m_d[j])
                blk_m_sb.append(t_)

            ic0a = singles.tile([128, NT * SH], dt)   # (s,h) per tile
            ic0b = singles.tile([128, NT * SH], dt)   # (h,s) per tile
            e_sb = singles.tile([128, NT * SE], dt)   # (i,j4) per tile
            usera = singles.tile([128, NG * 128], dt)  # group-major (t,h)
            ua = singles.tile([128, NG * 128], dt)     # u~ sample-major
            ud = singles.tile([128, NG * 128], F32R)   # u~ feature-major

            # ---- P0: user load + u~_0 = W_k @ user, per group ----
            for g in range(NG):
                src_ap = bass.AP(
                    tensor=user_d.tensor,
                    offset=user_d.offset + g * 1024 * H,
                    ap=[[H, 128], [128 * H, 8], [1, H]],
                )
                nc.sync.dma_start(out=usera[:, g * 128:(g + 1) * 128],
                                  in_=src_ap)
                tp = slots.get()
                nc.tensor.transpose(tp, usera[:, g * 128:(g + 1) * 128],
                                    ident)
                userd_g = scr.tile([128, 128], dt, name="userd_g", tag="gd0")
                nc.scalar.copy(userd_g, tp)
                up = slots.get()
                nc.tensor.matmul(up, _F(blk_wk_sb), userd_g,
                                 start=True, stop=True)
                nc.scalar.copy(_v(ud, g * 128, [[1, 128]]), up)
                tp2 = slots.get()
                nc.tensor.transpose(_R(tp2), _v(ud, g * 128, [[1, 128]]),
                                    identR)
                nc.scalar.copy(ua[:, g * 128:(g + 1) * 128], tp2)

            # ---- P1 emission quanta ----
            head = [True]

            def p1_tile(c):
                """DMA, d-major transpose, project, assemble one tile."""
                xstage = xst.tile([128, NSI * D], F32R, name="xstage")
                nc.sync.dma_start(out=xstage,
                                  in_=x_d[c * 128:(c + 1) * 128, :, :])
                xt_t = xtp.tile([D, NSI * 128], dt, name="xt_t")
                for gi, (s0, ns_) in enumerate(((0, 4), (4, 4), (8, 2))):
                    tp = ptx.tile([D, 512], dt, name="tpx", tag="tpx")
                    for k in range(ns_):
                        s = s0 + k
                        nc.tensor.transpose(
                            _R(tp[:, k * 128:(k + 1) * 128]),
                            xstage[:, s * D:(s + 1) * D],
                            identR)
                    dst = _v(xt_t, s0 * 128, [[1, ns_ * 128]], nparts=D)
                    srcv = _v(tp, 0, [[1, ns_ * 128]], nparts=D)
                    if gi == 2 or head[0]:
                        nc.vector.tensor_copy(dst, srcv)
                    else:
                        nc.scalar.copy(dst, srcv)
                # projection: xt chunks stationary, waug moving -> sample-major
                pp = pbig.tile([128, NSI * (H + NE)], dt, name="pp",
                               tag="pbig")
                for i in range(NSI):
                    nc.tensor.matmul(
                        _v(pp, i * (H + NE), [[1, H + NE]]),
                        xt_t[:, i * 128:(i + 1) * 128], waug_sb,
                        start=True, stop=True)
                base = c * SH
                ebase = c * SE
                nc.scalar.copy(
                    _v(ic0a, base, [[H, NSI], [1, H]]),
                    _v(pp, 0, [[H + NE, NSI], [1, H]]))
                nc.scalar.copy(
                    _v(e_sb, ebase, [[NE, NSI], [1, NE]]),
                    _v(pp, H, [[H + NE, NSI], [1, NE]]))
                nc.gpsimd.tensor_copy(
                    _v(ic0b, base, [[NSI, H], [1, NSI]]),
                    _v(ic0a, base, [[1, H], [H, NSI]]))

            # ---- P2: one interleaved iteration step for a group pair ----
            def p2_jstep(gpair, j, vouts, gd_pair):
                gas = []
                sels = []
                for gi_, g in enumerate(gpair):
                    prod_eng = nc.vector if gi_ == 0 else nc.gpsimd
                    sel_eng = nc.gpsimd if gi_ == 0 else nc.vector
                    base = g * 8 * SH
                    ua_sl = _v(ua, g * 128, [[H, 8], [0, NSI], [1, H]])
                    prod = scrbig.tile([128, 8, NSI, H], dt, name="prod",
                                       tag=f"prod{gi_}")
                    prod_eng.tensor_tensor(
                        out=prod,
                        in0=_v(ic0a, base, [[SH, 8], [H, NSI], [1, H]]),
                        in1=ua_sl, op=mybir.AluOpType.mult)
                    scores = scr.tile([128, 8, NSI], dt, name="scores",
                                      tag=f"scores{gi_}")
                    nc.vector.reduce_sum(out=scores, in_=prod,
                                         axis=mybir.AxisListType.X)
                    if j > 0:
                        scores2 = scr.tile([128, 8, NSI], dt, name="scores2",
                                           tag=f"scores2{gi_}")
                        nc.vector.tensor_tensor(
                            out=scores2, in0=scores,
                            in1=_v(e_sb, g * 8 * SE + (j - 1),
                                   [[SE, 8], [NE, NSI]]),
                            op=mybir.AluOpType.add)
                        scores = scores2
                    mx = scr.tile([128, 8], dt, name="mx", tag=f"mx{gi_}")
                    nc.vector.reduce_max(out=mx, in_=scores,
                                         axis=mybir.AxisListType.X)
                    mask = scr.tile([128, 8, NSI], dt, name="mask",
                                    tag=f"mask{gi_}")
                    nc.vector.tensor_tensor(
                        out=mask, in0=scores,
                        in1=_v(mx, 0, [[1, 8], [0, NSI]]),
                        op=mybir.AluOpType.is_equal)
                    sel = scrbig.tile([128, 8, H, NSI], dt, name="sel",
                                      tag=f"sel{gi_}")
                    sel_eng.tensor_tensor(
                        out=sel,
                        in0=_v(ic0b, base, [[SH, 8], [NSI, H], [1, NSI]]),
                        in1=_v(mask, 0, [[NSI, 8], [0, H], [1, NSI]]),
                        op=mybir.AluOpType.mult)
                    sels.append(sel)
                for gi_ in range(2):
                    ga = scr.tile([128, 8, H], dt, name="ga", tag=f"ga{gi_}")
                    nc.vector.reduce_sum(out=ga, in_=sels[gi_],
                                         axis=mybir.AxisListType.X)
                    gas.append(ga)
                for gi_ in range(2):
                    tpg = slots.get()
                    nc.tensor.transpose(tpg, gas[gi_], ident)
                    nc.scalar.copy(
                        _R(_v(gd_pair, gi_ * 128, [[1, 128]])), _R(tpg))
                p0 = gpair[0]
                ud2 = _v(ud, p0 * 128, [[1, 256]])
                # u~ recurrence first: cross-iteration critical path
                if j < SEARCH_NUM - 1:
                    up2 = slots.get(2)
                    nc.tensor.matmul(up2, blk_wk_sb, ud2,
                                     start=True, stop=False)
                    nc.tensor.matmul(up2, blk_m_sb[j], gd_pair,
                                     start=False, stop=True)
                    nc.scalar.copy(ud2, up2)
                    for gi_, g in enumerate(gpair):
                        tpu = slots.get()
                        nc.tensor.transpose(
                            _R(tpu), _v(ud, g * 128, [[1, 128]]), identR)
                        nc.scalar.copy(ua[:, g * 128:(g + 1) * 128], tpu)
                vp2 = slots.get(2)
                nc.tensor.matmul(vp2, blk_a_sb[j], gd_pair,
                                 start=True, stop=True)
                for gi_, g in enumerate(gpair):
                    vtmp = scr.tile([128, 128], dt, name="vtmp",
                                    tag=f"vtmp{gi_}")
                    nc.vector.tensor_scalar_add(
                        vtmp, _v(vp2, gi_ * 128, [[1, 128]]), cv_sb[j])
                    tpv = slots.get()
                    nc.tensor.transpose(tpv, vtmp, ident)
                    nc.scalar.copy(
                        _v(vouts[gi_], (1 + j) * H, [[96, 8], [1, H]]), tpv)

            def p2_finish(gpair, vouts):
                for gi_, g in enumerate(gpair):
                    nc.scalar.copy(_v(vouts[gi_], 0, [[96, 8], [1, H]]),
                                   _v(usera, g * 128, [[H, 8], [1, H]]))
                    for t in range(8):
                        c = g * 8 + t
                        nc.sync.dma_start(
                            out=out_d[c * 128:(c + 1) * 128, :, :],
                            in_=vouts[gi_][:, t * 96:(t + 1) * 96].rearrange(
                                "p (j h) -> p j h", j=6))

            # ---- software-pipelined emission ----
            def window_quanta(w):
                """P1 emission quanta for window w (16 tiles)."""
                return [lambda c=c: p1_tile(c)
                        for c in range(16 * w, 16 * w + 16)]

            for w in range(NW + 1):
                quanta = window_quanta(w) if w < NW else []
                if w == 0:
                    for q in quanta:
                        q()
                    head[0] = False
                    continue
                gpair = (2 * (w - 1), 2 * (w - 1) + 1)
                vouts = [vop.tile([128, 8 * 96], dt, name=f"vout{i}",
                                  tag=f"vout{i}") for i in range(2)]
                gd_pair = scr.tile([128, 256], F32R, name="gd_pair",
                                   tag="gdp")
                nq = len(quanta)
                cuts = [round(i * nq / SEARCH_NUM)
                        for i in range(SEARCH_NUM + 1)]
                for j in range(SEARCH_NUM):
                    for q in quanta[cuts[j]:cuts[j + 1]]:
                        q()
                    p2_jstep(gpair, j, vouts, gd_pair)
                p2_finish(gpair, vouts)


def _in_maps(inputs, B_core):
    cst = _host_constants(inputs["W_proj"], inputs["b_proj"],
                          inputs["W_k"], inputs["b_k"])
    x = np.ascontiguousarray(inputs["item_corpus"], dtype=np.float32)
    u = np.ascontiguousarray(inputs["user_intent"], dtype=np.float32)
    maps = []
    for core in range(NCORES):
        lo, hi = core * B_core, (core + 1) * B_core
        m = {"x": x[lo:hi], "user": u[lo:hi], "waug": cst["waug"],
             "blk_wk": cst["blk_wk"]}
        for j in range(SEARCH_NUM):
            m[f"blk_a{j}"] = cst[f"blk_a{j}"]
            m[f"cv{j}"] = cst[f"cv{j}"]
            if j < SEARCH_NUM - 1:
                m[f"blk_m{j}"] = cst[f"blk_m{j}"]
        maps.append(m)
    return maps


_COMPILED = {}


def _get_nc(B_core):
    if B_core not in _COMPILED:
        nc = bacc.Bacc("TRN2", target_bir_lowering=False, debug=False,
                       num_devices=NCORES)
        build_program(nc, B_core)
        nc.compile()
        _COMPILED[B_core] = nc
    return _COMPILED[B_core]


def kernel(**inputs) -> np.ndarray:
    bs = inputs["user_intent"].shape[0]
    assert bs % NCORES == 0
    B_core = bs // NCORES
    nc = _get_nc(B_core)
    res = run_bass_kernel_spmd(nc, _in_maps(inputs, B_core),
                               core_ids=list(range(NCORES)))
    out = np.concatenate([r["out"] for r in res.results], axis=0)
    return out.astype(np.float32)


# revision 22
# speedup vs baseline: 1.1857x; 1.1857x over previous
"""Trainium2 Bass kernel for nn_AttnGreedySearch (attn greedy top-1 search).

Math restructure (exact in exact arithmetic):
  With A_t = W_k^t and c_t = b_k @ sum_{i<t} W_k^i (row form), the iterated
  corpus is ic_t = ic0 @ A_t + c_t where ic0 = X @ W_proj + b_proj.  The
  device works with the bias-free ic0' = X @ W_proj and a DEFICIT-FREE u~
  recurrence u~_{j+1} = W_k u~_j + M_j g_j (no constant add): the dropped
  constants dv'_j accumulate into a sample-independent deficit
  D_{j+1} = W_k D_j + dv'_j whose score contribution <x W_proj, D_j> is
  emitted by the SAME projection matmul through 4 spare stationary columns
  (e_j), added to the scores before the argmax.  b_proj similarly folds into
  cv'_j / dv'_j.  Per iteration j:
      score_j[s,i] = <ic0'[s,i,:], u~_j> + e_j[s,i]   (argmax-equivalent)
      g_j = ic0'[argmax]                               (one-hot select)
      v_j = A_{j+1}^T g_j + cv'_j                      (output row, exact)
      u~_{j+1} = W_k u~_j + M_j g_j,   M_j = A_{j+2} A_{j+1}^T
  The 262MB corpus is read once, compressed by the projection, and the
  whole recurrence runs on 16-dim per-sample state.

Device dataflow (per core, batch B; super-tiles ST of 512 samples):
  P1 per tile: DMA X [128,1000]; PE-transpose item blocks -> psum; fat
      strided copies assemble d-major xt_sb [100, 10*512] (ACT+DVE).
  P1b per ST: projection as N=512 fp32 matmuls (M=32: 16 ic0 cols + 4
      e-cols), 4 items per psum bank at 32-partition offsets; copy
      psum->SBUF (ACT); PE-transpose [128,128] chunks back to sample-major;
      4-level-AP copies assemble ic0a (ACT), e_sb (ACT), and ic0b derived
      from ic0a on GPSIMD (SBUF->SBUF).
  P2: groups of 1024 samples processed in PAIRS with iterations interleaved:
      score = mul+grouped-reduce + e-add (DVE), one-hot mask (DVE), masked
      select-sum (GPSIMD mul + DVE reduce), 16x16 recurrence as pair-batched
      N=256 float32r block-diag matmuls (PE); copies on ACT.  P1 work is
      software-pipelined between P2 iteration steps at emission level so
      every engine FIFO alternates P1/P2 work.
  P3: per tile, one [128,96] DMA stores [user | v_0..v_4] rows.
"""

import numpy as np

import concourse.bass as bass
import concourse.mybir as mybir
import concourse.tile as tile
from concourse import bacc
from concourse.bass_utils import run_bass_kernel_spmd
from concourse.masks import make_identity

F32 = mybir.dt.float32
F32R = mybir.dt.float32r
SEARCH_NUM = 5
NCORES = 8
D = 100   # item feature dim
NSI = 10  # items per sample
H = 16    # projected dim
SH = NSI * H  # 160
NE = SEARCH_NUM - 1  # e-correction columns (j = 1..4)
SE = NSI * NE  # 40


def _host_constants(W_proj, b_proj, W_k, b_k):
    Wk = W_k.astype(np.float64)
    bk = b_k.astype(np.float64)
    bp = b_proj.astype(np.float64)
    Wp = W_proj.astype(np.float64)
    A = [np.eye(H)]
    for _ in range(SEARCH_NUM + 1):
        A.append(A[-1] @ Wk)
    c = [np.zeros(H)]
    for _ in range(SEARCH_NUM + 1):
        c.append(c[-1] @ Wk + bk)

    def blkdiag8(m):
        out = np.zeros((128, 128))
        for t in range(8):
            out[t * H:(t + 1) * H, t * H:(t + 1) * H] = m
        return out.astype(np.float32)

    cst = {"blk_wk": blkdiag8(Wk.T)}
    dvp = []
    for j in range(SEARCH_NUM):
        Aj1 = A[j + 1]
        cst[f"blk_a{j}"] = blkdiag8(Aj1)
        cv = bp @ Aj1 + c[j + 1]
        cst[f"cv{j}"] = np.tile(cv, 8).astype(np.float32)[:, None]
        if j < SEARCH_NUM - 1:
            Mj = A[j + 2] @ Aj1.T
            cst[f"blk_m{j}"] = blkdiag8(Mj.T)
            dvp.append(A[j + 2] @ c[j + 1] + Mj @ bp)
    # deficit recurrence D_{j+1} = Wk D_j + dv'_j  (D_0 = 0)
    Dd = [np.zeros(H)]
    for j in range(SEARCH_NUM - 1):
        Dd.append(Wk @ Dd[j] + dvp[j])
    waug = np.zeros((D, 32), dtype=np.float32)
    waug[:, :H] = W_proj.astype(np.float32)
    for j in range(1, SEARCH_NUM):
        waug[:, H + j - 1] = (Wp @ Dd[j]).astype(np.float32)
    cst["waug"] = waug
    return cst


def _v(t, off, dims, nparts=None):
    """View on tile/AP t: free dims `dims`, element offset `off` added.
    `nparts` overrides the partition count (step preserved)."""
    p = list(t.ap[0])
    if nparts is not None:
        p = [p[0], nparts]
    return bass.AP(tensor=t.tensor, offset=t.offset + off,
                   ap=[p] + [list(d) for d in dims])


def _R(x):
    """Bitcast an AP/tile to float32r."""
    if not isinstance(x, bass.AP):
        x = x.ap()
    return x.bitcast(F32R)


def _F(x):
    """Bitcast an AP/tile back to float32."""
    if not isinstance(x, bass.AP):
        x = x.ap()
    return x.bitcast(F32)


class _Slots:
    """Manual [128,128]-slot packing inside [128,512] PSUM bank tiles."""

    def __init__(self, pool, dt):
        self.pool, self.dt = pool, dt
        self.tile = None
        self.i = 4

    def get(self, w=1):
        if self.i + w > 4:
            self.tile = self.pool.tile([128, 512], self.dt, name="p2s",
                                       tag="p2s")
            self.i = 0
        v = _v(self.tile, self.i * 128, [[1, 128 * w]])
        self.i += w
        return v


def build_program(nc, B):
    assert B % 2048 == 0
    NT = B // 128
    NST = B // 512
    NG = B // 1024
    NW = NG // 2  # group pairs / pipeline windows
    dt = F32

    x_d = nc.dram_tensor("x", [B, NSI, D], F32R, kind="ExternalInput").ap()
    user_d = nc.dram_tensor("user", [B, H], dt, kind="ExternalInput").ap()
    waug_d = nc.dram_tensor("waug", [D, 32], dt, kind="ExternalInput").ap()
    blk_wk_d = nc.dram_tensor("blk_wk", [128, 128], F32R,
                              kind="ExternalInput").ap()
    blk_a_d = [nc.dram_tensor(f"blk_a{j}", [128, 128], F32R,
                              kind="ExternalInput").ap()
               for j in range(SEARCH_NUM)]
    cv_d = [nc.dram_tensor(f"cv{j}", [128, 1], dt, kind="ExternalInput").ap()
            for j in range(SEARCH_NUM)]
    blk_m_d = [nc.dram_tensor(f"blk_m{j}", [128, 128], F32R,
                              kind="ExternalInput").ap()
               for j in range(SEARCH_NUM - 1)]
    out_d = nc.dram_tensor("out", [B, SEARCH_NUM + 1, H], dt,
                           kind="ExternalOutput").ap()

    with tile.TileContext(nc) as tc:
        with tc.tile_pool(name="singles", bufs=1) as singles, \
             tc.tile_pool(name="xst", bufs=3) as xst, \
             tc.tile_pool(name="xtp", bufs=2) as xtp, \
             tc.tile_pool(name="pst", bufs=2) as pstp, \
             tc.tile_pool(name="scr", bufs=2) as scr, \
             tc.tile_pool(name="scrbig", bufs=1) as scrbig, \
             tc.tile_pool(name="vop", bufs=1) as vop, \
             tc.tile_pool(name="ptx", bufs=2, space="PSUM") as ptx, \
             tc.tile_pool(name="pbig", bufs=3, space="PSUM") as pbig, \
             tc.tile_pool(name="pp2", bufs=3, space="PSUM") as pp2:

            slots = _Slots(pp2, dt)

            # ---- persistent SBUF ----
            ident = singles.tile([128, 128], dt)
            make_identity(nc, ident)
            identR = singles.tile([128, 128], F32R)
            nc.scalar.copy(identR, ident)
            waug_sb = singles.tile([D, 32], dt)
            nc.sync.dma_start(out=waug_sb, in_=waug_d)
            blk_wk_sb = singles.tile([128, 128], F32R)
            nc.sync.dma_start(out=blk_wk_sb, in_=blk_wk_d)
            blk_a_sb, cv_sb, blk_m_sb = [], [], []
            for j in range(SEARCH_NUM):
                t_ = singles.tile([128, 128], F32R, name=f"blk_a{j}_sb")
                nc.sync.dma_start(out=t_, in_=blk_a_d[j])
                blk_a_sb.append(t_)
                t_ = singles.tile([128, 1], dt, name=f"cv{j}_sb")
                nc.sync.dma_start(out=t_, in_=cv_d[j])
                cv_sb.append(t_)
            for j in range(SEARCH_NUM - 1):
                t_ = singles.tile([128, 128], F32R, name=f"blk_m{j}_sb")
                nc.sync.dma_start(out=t_, in_=blk_m_d[j])
                blk_m_sb.append(t_)

            ic0a = singles.tile([128, NT * SH], dt)   # (s,h) per tile
            ic0b = singles.tile([128, NT * SH], dt)   # (h,s) per tile
            e_sb = singles.tile([128, NT * SE], dt)   # (i,j4) per tile
            usera = singles.tile([128, NG * 128], dt)  # group-major (t,h)
            ua = singles.tile([128, NG * 128], dt)     # u~ sample-major
            ud = singles.tile([128, NG * 128], F32R)   # u~ feature-major

            # ---- P0: user load + u~_0 = W_k @ user, per group ----
            for g in range(NG):
                src_ap = bass.AP(
                    tensor=user_d.tensor,
                    offset=user_d.offset + g * 1024 * H,
                    ap=[[H, 128], [128 * H, 8], [1, H]],
                )
                nc.sync.dma_start(out=usera[:, g * 128:(g + 1) * 128],
                                  in_=src_ap)
                tp = slots.get()
                nc.tensor.transpose(tp, usera[:, g * 128:(g + 1) * 128],
                                    ident)
                userd_g = scr.tile([128, 128], dt, name="userd_g", tag="gd0")
                nc.scalar.copy(userd_g, tp)
                up = slots.get()
                nc.tensor.matmul(up, _F(blk_wk_sb), userd_g,
                                 start=True, stop=True)
                nc.scalar.copy(_v(ud, g * 128, [[1, 128]]), up)
                tp2 = slots.get()
                nc.tensor.transpose(_R(tp2), _v(ud, g * 128, [[1, 128]]),
                                    identR)
                nc.scalar.copy(ua[:, g * 128:(g + 1) * 128], tp2)

            # ---- P1 emission quanta ----
            head = [True]

            def p1_tile(c):
                """DMA + d-major transpose of one 128-sample tile."""
                xstage = xst.tile([128, NSI * D], F32R, name="xstage")
                nc.sync.dma_start(out=xstage,
                                  in_=x_d[c * 128:(c + 1) * 128, :, :])
                a = c % 4
                xt_sb = xt_cur[0]
                for gi, (s0, ns_) in enumerate(((0, 4), (4, 4), (8, 2))):
                    tp = ptx.tile([D, 512], dt, name="tpx", tag="tpx")
                    for k in range(ns_):
                        s = s0 + k
                        nc.tensor.transpose(
                            _R(tp[:, k * 128:(k + 1) * 128]),
                            xstage[:, s * D:(s + 1) * D],
                            identR)
                    dst = _v(xt_sb, s0 * 512 + a * 128,
                             [[512, ns_], [1, 128]], nparts=D)
                    srcv = _v(tp, 0, [[128, ns_], [1, 128]])
                    if gi == 2 or (head[0] and gi == 1):
                        nc.vector.tensor_copy(dst, srcv)
                    else:
                        nc.scalar.copy(dst, srcv)

            def p1_trios(st):
                """Projection + layout flip back for one super-tile."""
                xt_sb = xt_cur[0]
                for items in ((0, 1, 2, 3), (4, 5, 6, 7), (8, 9)):
                    ni = len(items)
                    pp = pbig.tile([128, 512], dt, name="pp", tag="pbig")
                    for k, s in enumerate(items):
                        for a_mm in range(4):
                            out_mm = _v(pp, 32 * k * pp.ap[0][0] + a_mm * 128,
                                        [[1, 128]], nparts=32)
                            nc.tensor.matmul(
                                out_mm, waug_sb,
                                xt_sb[:, s * 512 + a_mm * 128:
                                      s * 512 + (a_mm + 1) * 128],
                                start=True, stop=True,
                                tile_position=(0, 32 * k))
                    pst = pstp.tile([128, 512], F32R, name="pst", tag="pst")
                    if head[0]:
                        nc.vector.tensor_copy(pst, pp)
                    else:
                        nc.scalar.copy(pst, pp)
                    tpb = pbig.tile([128, 512], dt, name="tpb", tag="pbig")
                    for a in range(4):
                        nc.tensor.transpose(
                            _R(tpb[:, a * 128:(a + 1) * 128]),
                            pst[:, a * 128:(a + 1) * 128], identR)
                    base = st * 4 * SH
                    ebase = st * 4 * SE
                    nc.scalar.copy(
                        _v(ic0a, base + items[0] * H,
                           [[SH, 4], [H, ni], [1, H]]),
                        _v(tpb, 0, [[128, 4], [32, ni], [1, H]]))
                    nc.scalar.copy(
                        _v(e_sb, ebase + items[0] * NE,
                           [[SE, 4], [NE, ni], [1, NE]]),
                        _v(tpb, H, [[128, 4], [32, ni], [1, NE]]))
                    nc.gpsimd.tensor_copy(
                        _v(ic0b, base + items[0],
                           [[SH, 4], [NSI, H], [1, ni]]),
                        _v(ic0a, base + items[0] * H,
                           [[SH, 4], [1, H], [H, ni]]))

            # ---- P2: one interleaved iteration step for a group pair ----
            def p2_jstep(gpair, j, vouts, gd_pair):
                gas = []
                sels = []
                for gi_, g in enumerate(gpair):
                    prod_eng = nc.vector if gi_ == 0 else nc.gpsimd
                    sel_eng = nc.gpsimd if gi_ == 0 else nc.vector
                    base = g * 8 * SH
                    ua_sl = _v(ua, g * 128, [[H, 8], [0, NSI], [1, H]])
                    prod = scrbig.tile([128, 8, NSI, H], dt, name="prod",
                                       tag=f"prod{gi_}")
                    prod_eng.tensor_tensor(
                        out=prod,
                        in0=_v(ic0a, base, [[SH, 8], [H, NSI], [1, H]]),
                        in1=ua_sl, op=mybir.AluOpType.mult)
                    scores = scr.tile([128, 8, NSI], dt, name="scores",
                                      tag=f"scores{gi_}")
                    nc.vector.reduce_sum(out=scores, in_=prod,
                                         axis=mybir.AxisListType.X)
                    if j > 0:
                        scores2 = scr.tile([128, 8, NSI], dt, name="scores2",
                                           tag=f"scores2{gi_}")
                        nc.vector.tensor_tensor(
                            out=scores2, in0=scores,
                            in1=_v(e_sb, g * 8 * SE + (j - 1),
                                   [[SE, 8], [NE, NSI]]),
                            op=mybir.AluOpType.add)
                        scores = scores2
                    mx = scr.tile([128, 8], dt, name="mx", tag=f"mx{gi_}")
                    nc.vector.reduce_max(out=mx, in_=scores,
                                         axis=mybir.AxisListType.X)
                    mask = scr.tile([128, 8, NSI], dt, name="mask",
                                    tag=f"mask{gi_}")
                    nc.vector.tensor_tensor(
                        out=mask, in0=scores,
                        in1=_v(mx, 0, [[1, 8], [0, NSI]]),
                        op=mybir.AluOpType.is_equal)
                    sel = scrbig.tile([128, 8, H, NSI], dt, name="sel",
                                      tag=f"sel{gi_}")
                    sel_eng.tensor_tensor(
                        out=sel,
                        in0=_v(ic0b, base, [[SH, 8], [NSI, H], [1, NSI]]),
                        in1=_v(mask, 0, [[NSI, 8], [0, H], [1, NSI]]),
                        op=mybir.AluOpType.mult)
                    sels.append(sel)
                for gi_ in range(2):
                    ga = scr.tile([128, 8, H], dt, name="ga", tag=f"ga{gi_}")
                    nc.vector.reduce_sum(out=ga, in_=sels[gi_],
                                         axis=mybir.AxisListType.X)
                    gas.append(ga)
                for gi_ in range(2):
                    tpg = slots.get()
                    nc.tensor.transpose(tpg, gas[gi_], ident)
                    nc.scalar.copy(
                        _R(_v(gd_pair, gi_ * 128, [[1, 128]])), _R(tpg))
                p0 = gpair[0]
                ud2 = _v(ud, p0 * 128, [[1, 256]])
                # u~ recurrence first: cross-iteration critical path
                if j < SEARCH_NUM - 1:
                    up2 = slots.get(2)
                    nc.tensor.matmul(up2, blk_wk_sb, ud2,
                                     start=True, stop=False)
                    nc.tensor.matmul(up2, blk_m_sb[j], gd_pair,
                                     start=False, stop=True)
                    nc.scalar.copy(ud2, up2)
                    for gi_, g in enumerate(gpair):
                        tpu = slots.get()
                        nc.tensor.transpose(
                            _R(tpu), _v(ud, g * 128, [[1, 128]]), identR)
                        nc.scalar.copy(ua[:, g * 128:(g + 1) * 128], tpu)
                vp2 = slots.get(2)
                nc.tensor.matmul(vp2, blk_a_sb[j], gd_pair,
                                 start=True, stop=True)
                for gi_, g in enumerate(gpair):
                    vtmp = scr.tile([128, 128], dt, name="vtmp",
                                    tag=f"vtmp{gi_}")
                    nc.vector.tensor_scalar_add(
                        vtmp, _v(vp2, gi_ * 128, [[1, 128]]), cv_sb[j])
                    tpv = slots.get()
                    nc.tensor.transpose(tpv, vtmp, ident)
                    nc.scalar.copy(
                        _v(vouts[gi_], (1 + j) * H, [[96, 8], [1, H]]), tpv)

            def p2_finish(gpair, vouts):
                for gi_, g in enumerate(gpair):
                    nc.scalar.copy(_v(vouts[gi_], 0, [[96, 8], [1, H]]),
                                   _v(usera, g * 128, [[H, 8], [1, H]]))
                    for t in range(8):
                        c = g * 8 + t
                        nc.sync.dma_start(
                            out=out_d[c * 128:(c + 1) * 128, :, :],
                            in_=vouts[gi_][:, t * 96:(t + 1) * 96].rearrange(
                                "p (j h) -> p j h", j=6))

            # ---- software-pipelined emission ----
            xt_cur = [None]

            def window_quanta(w):
                """P1 emission quanta for window w (4 super-tiles)."""
                quanta = []
                for st in range(4 * w, 4 * w + 4):
                    def new_xt(st=st):
                        xt_cur[0] = xtp.tile([D, NSI * 512], dt, name="xt_sb")
                    quanta.append(new_xt)
                    for a in range(4):
                        quanta.append(lambda c=st * 4 + a: p1_tile(c))
                    quanta.append(lambda st=st: p1_trios(st))
                return quanta

            for w in range(NW + 1):
                quanta = window_quanta(w) if w < NW else []
                if w == 0:
                    for q in quanta:
                        q()
                    head[0] = False
                    continue
                gpair = (2 * (w - 1), 2 * (w - 1) + 1)
                vouts = [vop.tile([128, 8 * 96], dt, name=f"vout{i}",
                                  tag=f"vout{i}") for i in range(2)]
                gd_pair = scr.tile([128, 256], F32R, name="gd_pair",
                                   tag="gdp")
                nq = len(quanta)
                cuts = [round(i * nq / SEARCH_NUM)
                        for i in range(SEARCH_NUM + 1)]
                for j in range(SEARCH_NUM):
                    for q in quanta[cuts[j]:cuts[j + 1]]:
                        q()
                    p2_jstep(gpair, j, vouts, gd_pair)
                p2_finish(gpair, vouts)


def _in_maps(inputs, B_core):
    cst = _host_constants(inputs["W_proj"], inputs["b_proj"],
                          inputs["W_k"], inputs["b_k"])
    x = np.ascontiguousarray(inputs["item_corpus"], dtype=np.float32)
    u = np.ascontiguousarray(inputs["user_intent"], dtype=np.float32)
    maps = []
    for core in range(NCORES):
        lo, hi = core * B_core, (core + 1) * B_core
        m = {"x": x[lo:hi], "user": u[lo:hi], "waug": cst["waug"],
             "blk_wk": cst["blk_wk"]}
        for j in range(SEARCH_NUM):
            m[f"blk_a{j}"] = cst[f"blk_a{j}"]
            m[f"cv{j}"] = cst[f"cv{j}"]
            if j < SEARCH_NUM - 1:
                m[f"blk_m{j}"] = cst[f"blk_m{j}"]
        maps.append(m)
    return maps


_COMPILED = {}


def _get_nc(B_core):
    if B_core not in _COMPILED:
        nc = bacc.Bacc("TRN2", target_bir_lowering=False, debug=False,
                       num_devices=NCORES)
        build_program(nc, B_core)
        nc.compile()
        _COMPILED[B_core] = nc
    return _COMPILED[B_core]


def kernel(**inputs) -> np.ndarray:
    bs = inputs["user_intent"].shape[0]
    assert bs % NCORES == 0
    B_core = bs // NCORES
    nc = _get_nc(B_core)
    res = run_bass_kernel_spmd(nc, _in_maps(inputs, B_core),
                               core_ids=list(range(NCORES)))
    out = np.concatenate([r["out"] for r in res.results], axis=0)
    return out.astype(np.float32)


# revision 27
# speedup vs baseline: 1.2024x; 1.0140x over previous
"""Trainium2 Bass kernel for nn_AttnGreedySearch (attn greedy top-1 search).

Math restructure (exact in exact arithmetic):
  With A_t = W_k^t and c_t = b_k @ sum_{i<t} W_k^i (row form), the iterated
  corpus is ic_t = ic0 @ A_t + c_t where ic0 = X @ W_proj + b_proj.  The
  device works with the bias-free ic0' = X @ W_proj and a DEFICIT-FREE u~
  recurrence u~_{j+1} = W_k u~_j + M_j g_j (no constant add): the dropped
  constants dv'_j accumulate into a sample-independent deficit
  D_{j+1} = W_k D_j + dv'_j whose score contribution <x W_proj, D_j> is
  emitted by the SAME projection matmul through 4 spare stationary columns
  (e_j), added to the scores before the argmax.  b_proj similarly folds into
  cv'_j.  Per iteration j:
      score_j[s,i] = <ic0'[s,i,:], u~_j> + e_j[s,i]   (argmax-equivalent)
      g_j = ic0'[argmax]                               (one-hot select)
      v_j = A_{j+1}^T g_j + cv'_j                      (output row, exact)
      u~_{j+1} = W_k u~_j + M_j g_j,   M_j = A_{j+2} A_{j+1}^T
  The 262MB corpus is read once, compressed by the projection, and the
  whole recurrence runs on 16-dim per-sample state.  float32r (TF32-like)
  is used on the PE for the x transposes and the block-diag recurrence
  matmuls; it costs a ~1e-6 relative truncation (a few dozen argmax flips
  out of 393k rows, rel-err ~8e-3, well inside the 2e-2 gate).

Device dataflow (per core, batch B; super-tiles ST of 512 samples):
  P1 per tile: DMA X [128,1000]; fp32r PE-transpose item blocks -> psum;
      fat strided copies assemble d-major xt_sb [100, 10*512] (ACT, DVE for
      the small chunk / head windows).
  P1b per ST: projection as N=512 fp32 matmuls (M=32: 16 ic0 cols + 4
      e-cols), 4 items per psum bank at 32-part offsets via col
      tile_position; copy psum->SBUF; fp32r PE-transpose [128,128] chunks
      back to sample-major; 4-level-AP copies assemble ic0a + e_sb (ACT)
      and ic0b ((h,i) order, GPSIMD SBUF->SBUF).
  P2: 1024-sample groups processed in PAIRS with iterations interleaved:
      per group score = mul+grouped-reduce + e-add, one-hot mask (DVE),
      masked select-sum with prod/sel split DVE<->GPSIMD across the pair,
      16x16 recurrence as pair-batched N=256 float32r block-diag matmuls
      (PE) with manual 4x[128,128] PSUM slot packing; copies on ACT.
      P1 work is software-pipelined between P2 iteration steps at emission
      level so every engine FIFO alternates P1/P2 work.
  P3: per tile, one [128,96] DMA stores [user | v_0..v_4] rows.

Measured on trn2: 487us/core (baseline handed to this session: 505us;
engine busy: PE ~300us, DVE ~270us, ACT ~195us, GPSIMD ~165us).
"""

import numpy as np

import concourse.bass as bass
import concourse.mybir as mybir
import concourse.tile as tile
from concourse import bacc
from concourse.bass_utils import run_bass_kernel_spmd
from concourse.masks import make_identity

F32 = mybir.dt.float32
F32R = mybir.dt.float32r
SEARCH_NUM = 5
NCORES = 8
D = 100   # item feature dim
NSI = 10  # items per sample
H = 16    # projected dim
SH = NSI * H  # 160
NE = SEARCH_NUM - 1  # e-correction columns (j = 1..4)
SE = NSI * NE  # 40


def _host_constants(W_proj, b_proj, W_k, b_k):
    Wk = W_k.astype(np.float64)
    bk = b_k.astype(np.float64)
    bp = b_proj.astype(np.float64)
    Wp = W_proj.astype(np.float64)
    A = [np.eye(H)]
    for _ in range(SEARCH_NUM + 1):
        A.append(A[-1] @ Wk)
    c = [np.zeros(H)]
    for _ in range(SEARCH_NUM + 1):
        c.append(c[-1] @ Wk + bk)

    def blkdiag8(m):
        out = np.zeros((128, 128))
        for t in range(8):
            out[t * H:(t + 1) * H, t * H:(t + 1) * H] = m
        return out.astype(np.float32)

    cst = {"blk_wk": blkdiag8(Wk.T)}
    dvp = []
    for j in range(SEARCH_NUM):
        Aj1 = A[j + 1]
        cst[f"blk_a{j}"] = blkdiag8(Aj1)
        cv = bp @ Aj1 + c[j + 1]
        cst[f"cv{j}"] = np.tile(cv, 8).astype(np.float32)[:, None]
        if j < SEARCH_NUM - 1:
            Mj = A[j + 2] @ Aj1.T
            cst[f"blk_m{j}"] = blkdiag8(Mj.T)
            dvp.append(A[j + 2] @ c[j + 1] + Mj @ bp)
    # deficit recurrence D_{j+1} = Wk D_j + dv'_j  (D_0 = 0)
    Dd = [np.zeros(H)]
    for j in range(SEARCH_NUM - 1):
        Dd.append(Wk @ Dd[j] + dvp[j])
    waug = np.zeros((D, 32), dtype=np.float32)
    waug[:, :H] = W_proj.astype(np.float32)
    for j in range(1, SEARCH_NUM):
        waug[:, H + j - 1] = (Wp @ Dd[j]).astype(np.float32)
    cst["waug"] = waug
    return cst


def _v(t, off, dims, nparts=None):
    """View on tile/AP t: free dims `dims`, element offset `off` added.
    `nparts` overrides the partition count (step preserved)."""
    p = list(t.ap[0])
    if nparts is not None:
        p = [p[0], nparts]
    return bass.AP(tensor=t.tensor, offset=t.offset + off,
                   ap=[p] + [list(d) for d in dims])


def _R(x):
    """Bitcast an AP/tile to float32r."""
    if not isinstance(x, bass.AP):
        x = x.ap()
    return x.bitcast(F32R)


def _F(x):
    """Bitcast an AP/tile back to float32."""
    if not isinstance(x, bass.AP):
        x = x.ap()
    return x.bitcast(F32)


class _Slots:
    """Manual [128,128]-slot packing inside [128,512] PSUM bank tiles."""

    def __init__(self, pool, dt):
        self.pool, self.dt = pool, dt
        self.tile = None
        self.i = 4

    def get(self, w=1):
        if self.i + w > 4:
            self.tile = self.pool.tile([128, 512], self.dt, name="p2s",
                                       tag="p2s")
            self.i = 0
        v = _v(self.tile, self.i * 128, [[1, 128 * w]])
        self.i += w
        return v


def build_program(nc, B):
    assert B % 2048 == 0
    NT = B // 128
    NST = B // 512
    NG = B // 1024
    NW = NG // 2  # group pairs / pipeline windows
    dt = F32

    x_d = nc.dram_tensor("x", [B, NSI, D], F32R, kind="ExternalInput").ap()
    user_d = nc.dram_tensor("user", [B, H], dt, kind="ExternalInput").ap()
    waug_d = nc.dram_tensor("waug", [D, 32], dt, kind="ExternalInput").ap()
    blk_wk_d = nc.dram_tensor("blk_wk", [128, 128], F32R,
                              kind="ExternalInput").ap()
    blk_a_d = [nc.dram_tensor(f"blk_a{j}", [128, 128], F32R,
                              kind="ExternalInput").ap()
               for j in range(SEARCH_NUM)]
    cv_d = [nc.dram_tensor(f"cv{j}", [128, 1], dt, kind="ExternalInput").ap()
            for j in range(SEARCH_NUM)]
    blk_m_d = [nc.dram_tensor(f"blk_m{j}", [128, 128], F32R,
                              kind="ExternalInput").ap()
               for j in range(SEARCH_NUM - 1)]
    out_d = nc.dram_tensor("out", [B, SEARCH_NUM + 1, H], dt,
                           kind="ExternalOutput").ap()

    with tile.TileContext(nc) as tc:
        with tc.tile_pool(name="singles", bufs=1) as singles, \
             tc.tile_pool(name="xst", bufs=2) as xst, \
             tc.tile_pool(name="xtp", bufs=2) as xtp, \
             tc.tile_pool(name="pst", bufs=2) as pstp, \
             tc.tile_pool(name="scr", bufs=2) as scr, \
             tc.tile_pool(name="scrbig", bufs=1) as scrbig, \
             tc.tile_pool(name="vop", bufs=1) as vop, \
             tc.tile_pool(name="ptx", bufs=2, space="PSUM") as ptx, \
             tc.tile_pool(name="pbig", bufs=3, space="PSUM") as pbig, \
             tc.tile_pool(name="pp2", bufs=3, space="PSUM") as pp2:

            slots = _Slots(pp2, dt)

            # ---- persistent SBUF ----
            ident = singles.tile([128, 128], dt)
            make_identity(nc, ident)
            identR = singles.tile([128, 128], F32R)
            nc.scalar.copy(identR, ident)
            waug_sb = singles.tile([D, 32], dt)
            nc.sync.dma_start(out=waug_sb, in_=waug_d)
            blk_wk_sb = singles.tile([128, 128], F32R)
            nc.sync.dma_start(out=blk_wk_sb, in_=blk_wk_d)
            blk_a_sb, cv_sb, blk_m_sb = [], [], []
            for j in range(SEARCH_NUM):
                t_ = singles.tile([128, 128], F32R, name=f"blk_a{j}_sb")
                nc.sync.dma_start(out=t_, in_=blk_a_d[j])
                blk_a_sb.append(t_)
                t_ = singles.tile([128, 1], dt, name=f"cv{j}_sb")
                nc.sync.dma_start(out=t_, in_=cv_d[j])
                cv_sb.append(t_)
            for j in range(SEARCH_NUM - 1):
                t_ = singles.tile([128, 128], F32R, name=f"blk_m{j}_sb")
                nc.sync.dma_start(out=t_, in_=blk_m_d[j])
                blk_m_sb.append(t_)

            ic0a = singles.tile([128, NT * SH], dt)   # (s,h) per tile
            ic0b = singles.tile([128, NT * SH], dt)   # (h,s) per tile
            e_sb = singles.tile([128, NT * SE], dt)   # (i,j4) per tile
            usera = singles.tile([128, NG * 128], dt)  # group-major (t,h)
            ua = singles.tile([128, NG * 128], dt)     # u~ sample-major
            ud = singles.tile([128, NG * 128], F32R)   # u~ feature-major

            # ---- P0: user load + u~_0 = W_k @ user, per group ----
            for g in range(NG):
                src_ap = bass.AP(
                    tensor=user_d.tensor,
                    offset=user_d.offset + g * 1024 * H,
                    ap=[[H, 128], [128 * H, 8], [1, H]],
                )
                nc.sync.dma_start(out=usera[:, g * 128:(g + 1) * 128],
                                  in_=src_ap)
                tp = slots.get()
                nc.tensor.transpose(tp, usera[:, g * 128:(g + 1) * 128],
                                    ident)
                userd_g = scr.tile([128, 128], dt, name="userd_g", tag="gd0")
                nc.scalar.copy(userd_g, tp)
                up = slots.get()
                nc.tensor.matmul(up, _F(blk_wk_sb), userd_g,
                                 start=True, stop=True)
                nc.scalar.copy(_v(ud, g * 128, [[1, 128]]), up)
                tp2 = slots.get()
                nc.tensor.transpose(_R(tp2), _v(ud, g * 128, [[1, 128]]),
                                    identR)
                nc.scalar.copy(ua[:, g * 128:(g + 1) * 128], tp2)

            # ---- P1 emission quanta ----
            head = [True]

            def p1_tile(c):
                """DMA + d-major transpose of one 128-sample tile."""
                xstage = xst.tile([128, NSI * D], F32R, name="xstage")
                nc.sync.dma_start(out=xstage,
                                  in_=x_d[c * 128:(c + 1) * 128, :, :])
                a = c % 4
                xt_sb = xt_cur[0]
                for gi, (s0, ns_) in enumerate(((0, 4), (4, 4), (8, 2))):
                    tp = ptx.tile([D, 512], dt, name="tpx", tag="tpx")
                    for k in range(ns_):
                        s = s0 + k
                        nc.tensor.transpose(
                            _R(tp[:, k * 128:(k + 1) * 128]),
                            xstage[:, s * D:(s + 1) * D],
                            identR)
                    dst = _v(xt_sb, s0 * 512 + a * 128,
                             [[512, ns_], [1, 128]], nparts=D)
                    srcv = _v(tp, 0, [[128, ns_], [1, 128]])
                    if gi == 2 or (head[0] and gi == 1):
                        nc.vector.tensor_copy(dst, srcv)
                    else:
                        nc.scalar.copy(dst, srcv)

            def p1_trios(st):
                """Projection + layout flip back for one super-tile."""
                xt_sb = xt_cur[0]
                for items in ((0, 1, 2, 3), (4, 5, 6, 7), (8, 9)):
                    ni = len(items)
                    pp = pbig.tile([128, 512], dt, name="pp", tag="pbig")
                    for k, s in enumerate(items):
                        for a_mm in range(4):
                            out_mm = _v(pp, 32 * k * pp.ap[0][0] + a_mm * 128,
                                        [[1, 128]], nparts=32)
                            nc.tensor.matmul(
                                out_mm, waug_sb,
                                xt_sb[:, s * 512 + a_mm * 128:
                                      s * 512 + (a_mm + 1) * 128],
                                start=True, stop=True,
                                tile_position=(0, 32 * k))
                    pst = pstp.tile([128, 512], F32R, name="pst", tag="pst")
                    if head[0]:
                        nc.vector.tensor_copy(pst, pp)
                    else:
                        nc.scalar.copy(pst, pp)
                    tpb = pbig.tile([128, 512], dt, name="tpb", tag="pbig")
                    for a in range(4):
                        nc.tensor.transpose(
                            _R(tpb[:, a * 128:(a + 1) * 128]),
                            pst[:, a * 128:(a + 1) * 128], identR)
                    base = st * 4 * SH
                    ebase = st * 4 * SE
                    nc.scalar.copy(
                        _v(ic0a, base + items[0] * H,
                           [[SH, 4], [H, ni], [1, H]]),
                        _v(tpb, 0, [[128, 4], [32, ni], [1, H]]))
                    nc.scalar.copy(
                        _v(e_sb, ebase + items[0] * NE,
                           [[SE, 4], [NE, ni], [1, NE]]),
                        _v(tpb, H, [[128, 4], [32, ni], [1, NE]]))
                    nc.gpsimd.tensor_copy(
                        _v(ic0b, base + items[0],
                           [[SH, 4], [NSI, H], [1, ni]]),
                        _v(ic0a, base + items[0] * H,
                           [[SH, 4], [1, H], [H, ni]]))

            # ---- P2: one iteration step for a group (two 512-halves) ----
            def p2_jstep(g, j, vout, gd_g):
                gas = []
                for hf in range(2):
                    prod_eng = nc.vector if hf == 0 else nc.gpsimd
                    sel_eng = nc.gpsimd if hf == 0 else nc.vector
                    base = g * 8 * SH + hf * 4 * SH
                    ua_sl = _v(ua, g * 128 + hf * 64,
                               [[H, 4], [0, NSI], [1, H]])
                    prod = scrbig.tile([128, 4, NSI, H], dt, name="prod",
                                       tag=f"prod{hf}")
                    prod_eng.tensor_tensor(
                        out=prod,
                        in0=_v(ic0a, base, [[SH, 4], [H, NSI], [1, H]]),
                        in1=ua_sl, op=mybir.AluOpType.mult)
                    scores = scr.tile([128, 4, NSI], dt, name="scores",
                                      tag=f"scores{hf}")
                    nc.vector.reduce_sum(out=scores, in_=prod,
                                         axis=mybir.AxisListType.X)
                    if j > 0:
                        scores2 = scr.tile([128, 4, NSI], dt, name="scores2",
                                           tag=f"scores2{hf}")
                        nc.vector.tensor_tensor(
                            out=scores2, in0=scores,
                            in1=_v(e_sb, g * 8 * SE + hf * 4 * SE + (j - 1),
                                   [[SE, 4], [NE, NSI]]),
                            op=mybir.AluOpType.add)
                        scores = scores2
                    mx = scr.tile([128, 4], dt, name="mx", tag=f"mx{hf}")
                    nc.vector.reduce_max(out=mx, in_=scores,
                                         axis=mybir.AxisListType.X)
                    mask = scr.tile([128, 4, NSI], dt, name="mask",
                                    tag=f"mask{hf}")
                    nc.vector.tensor_tensor(
                        out=mask, in0=scores,
                        in1=_v(mx, 0, [[1, 4], [0, NSI]]),
                        op=mybir.AluOpType.is_equal)
                    sel = scrbig.tile([128, 4, H, NSI], dt, name="sel",
                                      tag=f"sel{hf}")
                    sel_eng.tensor_tensor(
                        out=sel,
                        in0=_v(ic0b, base, [[SH, 4], [NSI, H], [1, NSI]]),
                        in1=_v(mask, 0, [[NSI, 4], [0, H], [1, NSI]]),
                        op=mybir.AluOpType.mult)
                    gas.append(sel)
                ga = scr.tile([128, 8, H], dt, name="ga", tag="ga")
                for hf in range(2):
                    nc.vector.reduce_sum(
                        out=_v(ga, hf * 64, [[H, 4], [1, H]]),
                        in_=gas[hf], axis=mybir.AxisListType.X)
                tpg = slots.get()
                nc.tensor.transpose(tpg, ga, ident)
                nc.scalar.copy(gd_g, tpg)
                ud1 = _v(ud, g * 128, [[1, 128]])
                # u~ recurrence first: cross-iteration critical path
                if j < SEARCH_NUM - 1:
                    up = slots.get()
                    nc.tensor.matmul(up, blk_wk_sb, ud1,
                                     start=True, stop=False)
                    nc.tensor.matmul(up, blk_m_sb[j], gd_g,
                                     start=False, stop=True)
                    nc.scalar.copy(ud1, up)
                    tpu = slots.get()
                    nc.tensor.transpose(_R(tpu), ud1, identR)
                    nc.scalar.copy(ua[:, g * 128:(g + 1) * 128], tpu)
                vp = slots.get()
                nc.tensor.matmul(vp, blk_a_sb[j], gd_g,
                                 start=True, stop=True)
                vtmp = scr.tile([128, 128], dt, name="vtmp", tag="vtmp")
                nc.vector.tensor_scalar_add(vtmp, vp, cv_sb[j])
                tpv = slots.get()
                nc.tensor.transpose(tpv, vtmp, ident)
                nc.scalar.copy(_v(vout, (1 + j) * H, [[96, 8], [1, H]]), tpv)

            def p2_finish(g, vout):
                nc.scalar.copy(_v(vout, 0, [[96, 8], [1, H]]),
                               _v(usera, g * 128, [[H, 8], [1, H]]))
                for t in range(8):
                    c = g * 8 + t
                    nc.sync.dma_start(
                        out=out_d[c * 128:(c + 1) * 128, :, :],
                        in_=vout[:, t * 96:(t + 1) * 96].rearrange(
                            "p (j h) -> p j h", j=6))

            # ---- software-pipelined emission ----
            xt_cur = [None]

            def window_quanta(w):
                """P1 emission quanta for window w (2 super-tiles)."""
                quanta = []
                for st in range(2 * w, 2 * w + 2):
                    def new_xt(st=st):
                        xt_cur[0] = xtp.tile([D, NSI * 512], dt, name="xt_sb")
                    quanta.append(new_xt)
                    for a in range(4):
                        quanta.append(lambda c=st * 4 + a: p1_tile(c))
                    quanta.append(lambda st=st: p1_trios(st))
                return quanta

            state = {}

            def get_state(g):
                if g not in state:
                    state[g] = (
                        vop.tile([128, 8 * 96], dt, name="vout",
                                 tag=f"vout{g % 2}"),
                        scr.tile([128, 128], F32R, name="gd_g",
                                 tag=f"gdp{g % 2}"),
                    )
                return state[g]

            NWIN = NG  # 8 P1 windows of 2 STs
            for w in range(NWIN + 2):
                quanta = window_quanta(w) if w < NWIN else []
                # P2 steps this window: late steps of group w-2, early of w-1
                steps = []
                for (g, js) in ((w - 2, (2, 3, 4)), (w - 1, (0, 1))):
                    if 0 <= g < NG:
                        steps.extend((g, j) for j in js)
                if w == 0:
                    for q in quanta:
                        q()
                    head[0] = False
                    continue
                ns_ = max(len(steps), 1)
                nq = len(quanta)
                cuts = [round(i * nq / ns_) for i in range(ns_ + 1)]
                for i, (g, j) in enumerate(steps):
                    for q in quanta[cuts[i]:cuts[i + 1]]:
                        q()
                    vout, gd_g = get_state(g)
                    p2_jstep(g, j, vout, gd_g)
                    if j == SEARCH_NUM - 1:
                        p2_finish(g, vout)
                        del state[g]
                for q in quanta[cuts[len(steps)]:] if steps else quanta:
                    q()
